# revision 1
# baseline (speedup 1.0000x reference)
"""Bass/Tile multi-head attention kernel builder for TRN2.

Per-core problem (core c handles batch b=c//2, head-group g=c%2):
  inputs:  xq, xk, xv [S, DIN] f32      (batch b slices of q/k/v)
           wq, wk, wv [DIN, DC] f32     (column slice for this head group)
           wo [DC, DOUT] f32            (row slice)
           bq, bk, bv [DC] f32
  output:  out [S, DOUT] f32  partial:  host sums the two head-group partials
           per batch and adds bo.

Math (per head h of H local heads, depth=64):
  QT = (xq @ wq + bq).T        [DC, S]   (d_core major)
  KT = (xk @ wk + bk).T        [DC, S]
  V  = xv @ wv + bv            [S, DC]   (+ ones column per head -> V_aug)
  ST_h = KT_h-slices.T @ QT_h  -> S^T tiles [sk, sq]
  E = exp(ST * 1/sqrt(depth))            (no row-max subtraction: logits are O(10))
  OT_aug = V_aug_h.T @ E       [65, sq]  (row 64 = softmax denominator)
  OTn_h = OT_aug[0:64] / OT_aug[64]      (normalized attention output, transposed)
  out = OTn.T @ wo                       (accumulated over all local heads)

Layouts (P=128 partitions):
  QT/KT: [128, DC//128, S]   d_core = blk*128 + p  (head h -> blk h//2, partitions (h%2)*64..)
  V:     [128, S//128, H, 65]  sk = chunk*128 + p; col 64 = 1.0
  OTn:   [128, DC//128, SQT]  same d_core layout as QT -> out-proj lhsT with K=128
"""

from contextlib import ExitStack

import concourse.mybir as mybir
from concourse import bacc
from concourse.masks import make_identity
from concourse.tile import TileContext

F32 = mybir.dt.float32
F32R = mybir.dt.float32r
P = 128
EXP = mybir.ActivationFunctionType.Exp


def build_mha_core(S=2048, DIN=1024, DC=512, DOUT=1024, H=8, depth=64,
                   SQT=512, KG=2, num_devices=1, ablate="", stage_bufs=2,
                   q_bufs=2, ex_bufs=2, xt_bufs=1, ot_bufs=1):
    ablate = set(ablate.split(",")) if ablate else set()
    assert DC == H * depth and DC % P == 0 and DIN % P == 0 and S % SQT == 0
    NKT = S // P          # key chunks of 128
    NDIN = DIN // P       # input-dim k-tiles
    NDO = DC // P         # d_core blocks
    NSQT = S // SQT       # attention q tiles
    NSUB = SQT // 256     # 256-row transpose chunks per sqt
    assert NKT % KG == 0
    scale = 1.0 / float(depth) ** 0.5

    nc = bacc.Bacc("TRN2", target_bir_lowering=False, debug=False,
                   num_devices=num_devices)
    xq = nc.dram_tensor("xq", [S, DIN], F32, kind="ExternalInput")
    xk = nc.dram_tensor("xk", [S, DIN], F32, kind="ExternalInput")
    xv = nc.dram_tensor("xv", [S, DIN], F32, kind="ExternalInput")
    wq = nc.dram_tensor("wq", [DIN, DC], F32, kind="ExternalInput")
    wk = nc.dram_tensor("wk", [DIN, DC], F32, kind="ExternalInput")
    wv = nc.dram_tensor("wv", [DIN, DC], F32, kind="ExternalInput")
    wo = nc.dram_tensor("wo", [DC, DOUT], F32, kind="ExternalInput")
    bq = nc.dram_tensor("bq", [DC], F32, kind="ExternalInput")
    bk = nc.dram_tensor("bk", [DC], F32, kind="ExternalInput")
    bv = nc.dram_tensor("bv", [DC], F32, kind="ExternalInput")
    out = nc.dram_tensor("out", [S, DOUT], F32, kind="ExternalOutput")
    dbg = "dbg" in ablate
    if dbg:
        d_qt = nc.dram_tensor("d_qt", [P, (DC // P) * SQT], F32, kind="ExternalOutput")
        d_kt = nc.dram_tensor("d_kt", [P, (DC // P) * 512], F32, kind="ExternalOutput")
        d_v = nc.dram_tensor("d_v", [P, H * (depth + 1)], F32, kind="ExternalOutput")
        d_ex = nc.dram_tensor("d_ex", [P, KG * 512], F32, kind="ExternalOutput")
        d_ot = nc.dram_tensor("d_ot", [depth + 1, SQT], F32, kind="ExternalOutput")
        d_otn = nc.dram_tensor("d_otn", [P, (DC // P) * SQT], F32, kind="ExternalOutput")

    with TileContext(nc) as tc, ExitStack() as ctx:
        # pools alive for the whole kernel
        const = ctx.enter_context(tc.tile_pool(name="const", bufs=1))
        wpool = ctx.enter_context(tc.tile_pool(name="wpool", bufs=1))
        kvpool = ctx.enter_context(tc.tile_pool(name="kv", bufs=1))
        stage = ctx.enter_context(tc.tile_pool(name="stage", bufs=stage_bufs))
        xtpool = ctx.enter_context(tc.tile_pool(name="xt", bufs=xt_bufs))
        ps_st = ctx.enter_context(tc.tile_pool(name="ps_st", bufs=1, space="PSUM"))
        ps_acc = ctx.enter_context(tc.tile_pool(name="ps_acc", bufs=1, space="PSUM"))
        ps_gen = ctx.enter_context(tc.tile_pool(name="ps_gen", bufs=2, space="PSUM"))

        ident = const.tile([P, P], F32)
        make_identity(nc, ident)
        ones_f = const.tile([P, 1], F32)
        nc.vector.memset(ones_f[:], 1.0)

        # ---- weight loading: stage in <=8KB/partition chunks, round to fp32r
        def load_weight(pool, dram, kdim, ndim, name):
            w = pool.tile([P, kdim // P, ndim], F32R, name=name)
            cblk = max(1, 2048 // ndim)  # din-blocks per staging chunk
            for c0 in range(0, kdim // P, cblk):
                c1 = min(c0 + cblk, kdim // P)
                st = stage.tile([P, cblk, 2048 // cblk], F32, tag="stage8",
                                name="wst")
                stv = st[:, :c1 - c0, :ndim]
                nc.sync.dma_start(
                    stv[:],
                    dram[c0 * P:c1 * P, :].rearrange("(o p) n -> p o n", p=P))
                nc.vector.tensor_copy(w[:, c0:c1, :], stv[:])
            return w

        bq_sb = const.tile([P, NDO], F32)
        nc.sync.dma_start(bq_sb[:], bq[:].rearrange("(o p) -> p o", p=P))
        bk_sb = const.tile([P, NDO], F32)
        nc.sync.dma_start(bk_sb[:], bk[:].rearrange("(o p) -> p o", p=P))
        bv_st = const.tile([1, DC], F32)
        nc.sync.dma_start(bv_st[0:1, :], bv[:][None, :])
        bv_bc = const.tile([P, DC], F32)
        nc.gpsimd.partition_broadcast(bv_bc[:], bv_st[0:1, :])

        # ---- transpose helper: x rows [r0, r0+256) -> xt[:, :, soff:soff+256]
        def transpose_chunk(xdram, r0, xt, soff, on_act=False):
            xn = stage.tile([P, 2, DIN], F32, tag="stage8", name="xn")
            nc.sync.dma_start(
                xn[:], xdram[r0:r0 + 256, :].rearrange("(c p) d -> p c d", p=P))
            for dblk in range(0 if "notr" in ablate else NDIN):
                tp = ps_gen.tile([P, 512], F32, tag="gen", name="tp")
                for sb in range(2):
                    nc.tensor.transpose(
                        tp[:, sb * P:(sb + 1) * P],
                        xn[:, sb, dblk * P:(dblk + 1) * P], ident[:])
                if on_act:
                    nc.scalar.copy(xt[:, dblk, soff:soff + 256], tp[:, 0:256])
                else:
                    nc.vector.tensor_copy(xt[:, dblk, soff:soff + 256],
                                          tp[:, 0:256])

        # ---- K/V phase (wk/wv live only here) ----
        V = kvpool.tile([P, NKT, H, depth + 1], F32R)
        KT = kvpool.tile([P, NDO, S], F32R)
        nc.vector.tensor_copy(
            V[:, :, :, depth:depth + 1],
            ones_f[:, None, None, 0:1].to_broadcast((P, NKT, H, 1)))

        with tc.tile_pool(name="wkv", bufs=1) as wkvpool, \
                tc.tile_pool(name="xtkv", bufs=2) as xtkv:
            wk_sb = load_weight(wkvpool, wk, DIN, DC, "wk_sb")
            wv_sb = load_weight(wkvpool, wv, DIN, DC, "wv_sb")

            for st_i in range(S // 512):
                xt = xtkv.tile([P, NDIN, 512], F32R, tag="xt", name="xt")
                for sub in range(2):
                    transpose_chunk(xk, st_i * 512 + sub * 256, xt, sub * 256,
                                    on_act=True)
                for do in range(NDO):
                    ps = ps_gen.tile([P, 512], F32, tag="gen", name="psk")
                    for kt in range(NDIN):
                        nc.tensor.matmul(
                            ps[:], wk_sb[:, kt, do * P:(do + 1) * P], xt[:, kt, :],
                            start=(kt == 0), stop=(kt == NDIN - 1))
                    nc.scalar.activation(
                        KT[:, do, st_i * 512:(st_i + 1) * 512], ps[:],
                        mybir.ActivationFunctionType.Identity,
                        bias=bk_sb[:, do:do + 1])

            for st_i in range(S // 512):
                xt = xtkv.tile([P, NDIN, 512], F32R, tag="xt", name="xt")
                for sub in range(2):
                    transpose_chunk(xv, st_i * 512 + sub * 256, xt, sub * 256,
                                    on_act=True)
                for sc in range(4):  # 128-row chunks
                    ps_full = ps_gen.tile([P, 512], F32, tag="gen", name="psv")
                    ps = ps_full[:, :DC]
                    for kt in range(NDIN):
                        nc.tensor.matmul(
                            ps[:], xt[:, kt, sc * P:(sc + 1) * P], wv_sb[:, kt, :],
                            start=(kt == 0), stop=(kt == NDIN - 1))
                    chunk = st_i * 4 + sc
                    nc.vector.tensor_tensor(
                        V[:, chunk, :, 0:depth],
                        ps[:].rearrange("p (h d) -> p h d", h=H),
                        bv_bc[:].rearrange("p (h d) -> p h d", h=H),
                        mybir.AluOpType.add)

        if dbg:
            nc.sync.dma_start(d_kt[:, :], KT[:, :, 0:512].bitcast(F32))
            nc.sync.dma_start(d_v[:, :], V[:, 0, :, :].bitcast(F32))

        # wq/wo loaded after wkv released
        wq_sb = load_weight(wpool, wq, DIN, DC, "wq_sb")
        wo_sb = load_weight(wpool, wo, DC, DOUT, "wo_sb")

        # ---- attention (+ pipelined Q-proj and out-proj) per sqt ----
        qpool = ctx.enter_context(tc.tile_pool(name="qp", bufs=q_bufs))
        otpool = ctx.enter_context(tc.tile_pool(name="ot", bufs=ot_bufs))
        expool = ctx.enter_context(tc.tile_pool(name="ex", bufs=ex_bufs))
        misc = ctx.enter_context(tc.tile_pool(name="misc", bufs=2))

        def qproj(sqt):
            xt = xtpool.tile([P, NDIN, SQT], F32R, tag="xt", name="xt")
            for sub in range(NSUB):
                transpose_chunk(xq, sqt * SQT + sub * 256, xt, sub * 256)
            QTe = qpool.tile([P, NDO, SQT], F32R, tag="qte", name="qte")
            QTo = qpool.tile([P, NDO, SQT], F32R, tag="qto", name="qto")
            for do in range(NDO):
                ps_full = ps_gen.tile([P, 512], F32, tag="gen", name="psq")
                ps = ps_full[:, :SQT]
                for kt in range(NDIN):
                    nc.tensor.matmul(
                        ps[:], wq_sb[:, kt, do * P:(do + 1) * P], xt[:, kt, :],
                        start=(kt == 0), stop=(kt == NDIN - 1))
                qb = misc.tile([P, SQT], F32R, tag="qb", name="qb", bufs=1)
                nc.vector.tensor_scalar_add(qb[:], ps[:], bq_sb[:, do:do + 1])
                nc.vector.tensor_copy(QTe[0:64, do, :], qb[0:64, :])
                nc.vector.tensor_copy(QTo[64:128, do, :], qb[64:128, :])
                nc.vector.memset(QTe[64:128, do, :].bitcast(F32), 0.0)
                nc.vector.memset(QTo[0:64, do, :].bitcast(F32), 0.0)
            return QTe, QTo

        NSQT_EFF = 0 if "kvonly" in ablate else NSQT
        QT_next = qproj(0) if NSQT_EFF else None
        for sqt in range(NSQT_EFF):
            QTe, QTo = QT_next
            if sqt + 1 < NSQT_EFF:
                QT_next = qproj(sqt + 1)

            if dbg and sqt == 0:
                nc.sync.dma_start(d_qt[:, :], QTe[:, :, :].bitcast(F32))
            OTn = otpool.tile([P, NDO, SQT], F32R, tag="otn", name="otn")
            if "noattn" in ablate:
                nc.vector.memset(OTn[:].bitcast(F32), 0.0)
            for hp in range(0 if "noattn" in ablate else H // 2):  # head pairs interleaved for PE row concurrency
                heads = (2 * hp, 2 * hp + 1)
                ot_ps = {}
                for h in heads:
                    ot_t = ps_acc.tile([depth + 1, SQT], F32, name=f"ot{h % 2}")
                    ot_ps[h] = ot_t
                for kg in range(NKT // KG):
                    st_ps = {}
                    for h in heads:
                        st_t = ps_st.tile([P, KG, 512], F32, name=f"st{h % 2}")
                        st_ps[h] = st_t
                    for j in range(KG):
                        kt = kg * KG + j
                        for h in heads:
                            blk = h // 2
                            qmask = QTe if h % 2 == 0 else QTo
                            nc.tensor.matmul(
                                st_ps[h][:, j],
                                KT[:, blk, kt * P:(kt + 1) * P],
                                qmask[:, blk, :],
                                start=True, stop=True)
                    ex = {}
                    for h in heads:
                        ex_t = expool.tile([P, KG, 512], F32R, tag=f"ex{h % 2}",
                                           name=f"ex{h % 2}")
                        ex[h] = ex_t
                        if "expdve" in ablate:
                            nc.vector.tensor_copy(ex_t[:], st_ps[h][:])
                        else:
                            nc.scalar.activation(ex_t[:], st_ps[h][:], EXP,
                                                 scale=scale)
                    if dbg and sqt == 0 and hp == 0 and kg == 0:
                        nc.sync.dma_start(d_ex[:, :], ex[0][:].bitcast(F32))
                    for j in range(KG):
                        kt = kg * KG + j
                        for h in heads:
                            nc.tensor.matmul(
                                ot_ps[h][:], V[:, kt, h, :], ex[h][:, j],
                                start=(kt == 0), stop=(kt == NKT - 1))
                if dbg and sqt == 0 and hp == 0:
                    otdump = misc.tile([depth + 1, SQT], F32, tag="otd",
                                       name="otdump")
                    nc.vector.tensor_copy(otdump[:], ot_ps[0][:])
                    nc.sync.dma_start(d_ot[:, :], otdump[:])
                for h in heads:
                    p0, blk = (h % 2) * 64, h // 2
                    if "nonorm" in ablate:
                        nc.vector.tensor_copy(OTn[p0:p0 + 64, blk, :],
                                              ot_ps[h][0:depth, :])
                        continue
                    # all compute at partition base 0; only the final
                    # plain tensor_copy (HW-proven base shifter) moves data
                    den = misc.tile([1, SQT], F32, tag="den", name="den", bufs=1)
                    nc.vector.tensor_copy(den[0:1, :],
                                          ot_ps[h][depth:depth + 1, :])
                    rec = misc.tile([1, SQT], F32, tag="rec", name="rec", bufs=1)
                    nc.vector.reciprocal(rec[0:1, :], den[0:1, :])
                    bc = misc.tile([64, SQT], F32, tag="bc", name="bc", bufs=1)
                    nc.gpsimd.partition_broadcast(bc[0:64, :], rec[0:1, :])
                    onorm = misc.tile([64, SQT], F32R, tag="onorm", name="onorm", bufs=1)
                    nc.vector.tensor_tensor(
                        onorm[0:64, :], ot_ps[h][0:depth, :],
                        bc[0:64, :], mybir.AluOpType.mult)
                    nc.vector.tensor_copy(OTn[p0:p0 + 64, blk, :],
                                          onorm[0:64, :])

            if dbg and sqt == 0:
                nc.sync.dma_start(d_otn[:, :], OTn[:, :, :].bitcast(F32))

            # out projection for this sqt
            DOW = min(512, DOUT)
            for do in range(DOUT // DOW):
                for sc in range(SQT // P):
                    ps_full = ps_gen.tile([P, 512], F32, tag="gen", name="pso")
                    ps = ps_full[:, :DOW]
                    for hh in range(NDO):
                        nc.tensor.matmul(
                            ps[:], OTn[:, hh, sc * P:(sc + 1) * P],
                            wo_sb[:, hh, do * DOW:(do + 1) * DOW],
                            start=(hh == 0), stop=(hh == NDO - 1))
                    osb = misc.tile([P, 512], F32, tag="osb", name="osb")
                    nc.vector.tensor_copy(osb[:, :DOW], ps[:])
                    r0 = sqt * SQT + sc * P
                    nc.sync.dma_start(out[r0:r0 + P, do * DOW:(do + 1) * DOW],
                                      osb[:, :DOW])

    nc.compile()
    return nc


# ---------------------------------------------------------------------------
# Host-side wrapper: shard across 8 NeuronCores, run SPMD, gather.
# Core c handles batch b = c // 2 and head-group g = c % 2 (8 of 16 heads,
# i.e. columns [g*512, (g+1)*512) of Wq/Wk/Wv and rows of Wo).
# ---------------------------------------------------------------------------

import numpy as np

from concourse.bass_utils import run_bass_kernel_spmd

_NC = None


def _get_nc():
    global _NC
    if _NC is None:
        _NC = build_mha_core(S=2048, DIN=1024, DC=512, DOUT=1024, H=8,
                             depth=64, num_devices=8)
    return _NC


def _in_maps(q, k, v, Wq, bq, Wk, bk, Wv, bv, Wo, bo):
    f32 = np.float32
    maps = []
    for c in range(8):
        b, g = c // 2, c % 2
        sl = slice(g * 512, (g + 1) * 512)
        maps.append({
            "xq": np.ascontiguousarray(q[b], dtype=f32),
            "xk": np.ascontiguousarray(k[b], dtype=f32),
            "xv": np.ascontiguousarray(v[b], dtype=f32),
            "wq": np.ascontiguousarray(Wq[:, sl], dtype=f32),
            "wk": np.ascontiguousarray(Wk[:, sl], dtype=f32),
            "wv": np.ascontiguousarray(Wv[:, sl], dtype=f32),
            "wo": np.ascontiguousarray(Wo[sl, :], dtype=f32),
            "bq": np.ascontiguousarray(bq[sl], dtype=f32),
            "bk": np.ascontiguousarray(bk[sl], dtype=f32),
            "bv": np.ascontiguousarray(bv[sl], dtype=f32),
        })
    return maps


def _gather(results, bo):
    out = np.empty((4, 2048, 1024), dtype=np.float32)
    bo32 = np.asarray(bo, dtype=np.float32)
    for b in range(4):
        out[b] = results[2 * b]["out"] + results[2 * b + 1]["out"] + bo32
    return out


def kernel(q, k, v, Wq, bq, Wk, bk, Wv, bv, Wo, bo, _trace=False):
    nc = _get_nc()
    res = run_bass_kernel_spmd(
        nc, _in_maps(q, k, v, Wq, bq, Wk, bk, Wv, bv, Wo, bo),
        core_ids=list(range(8)), trace=_trace)
    out = _gather(res.results, bo)
    if _trace:
        kernel.last_results = res
    return out



# revision 2
# speedup vs baseline: 1.2491x; 1.2491x over previous
"""Bass/Tile multi-head attention kernel builder for TRN2 (v2).

Per-core problem (core c handles batch b=c//2, head-group g=c%2):
  inputs:  xq, xk, xv [S, DIN] bf16     (batch b slices of q/k/v, host-cast)
           wq, wk, wv [DIN, DC] bf16    (column slice for this head group)
           wo [DC, DOUT] bf16           (row slice)
           bq, bk, bv [DC] f32
  output:  out [S, DOUT] f32  partial:  host sums the two head-group partials
           per batch and adds bo.

Math (per head h of H local heads, depth=64):
  xt   = X^T via DMA-xbar transpose loads          [DIN(p-major blocks), S]
  QT   = (wq_blk.T @ xt) + bq                       [DC, S]  f32r
  KT   = (wk_blk.T @ xt) + bk                       [DC, S]  f32r
  V    = (xt_chunk.T @ wv) + bv (+ ones col)        [S, DC(+1/head)] bf16
  ST   = KT_h.T @ QT_h   (64-partition contraction) [keys, q] per head
  E    = exp(ST * 1/sqrt(depth))  -> bf16           (logits O(10), no max-sub)
  OT   = E_chunk.T @ V_aug_h  (transposed-AV)       [q, depth+1] accum over keys
  O    = OT[:, :depth] / OT[:, depth]  -> bf16      (free-dim normalize)
  OTn  = O^T per head pair (PE transpose)           [DC, S] bf16
  out  = OTn.T @ wo                                 [S, DOUT] f32

Layouts (P=128 partitions):
  QT/KT: [128, DC//128, S]  d_core = blk*128 + p (head h -> blk h//2,
         partitions (h%2)*64 ..); scores contract over 64 partitions.
  V:     [128, S//128, H, 65]  key = chunk*128 + p; col 64 = 1.0
  OTn:   [128, DC//128, SQT]  same d_core layout -> out-proj lhsT with K=128
"""

from contextlib import ExitStack

import concourse.mybir as mybir
from concourse import bacc
from concourse.masks import make_identity
from concourse.tile import TileContext

F32 = mybir.dt.float32
F32R = mybir.dt.float32r
BF16 = mybir.dt.bfloat16
P = 128
EXP = mybir.ActivationFunctionType.Exp
IDENT = mybir.ActivationFunctionType.Identity


def build_mha_core(S=2048, DIN=1024, DC=512, DOUT=1024, H=8, depth=64,
                   SQT=512, num_devices=1, ablate="", q_bufs=2, ex_bufs=3,
                   st_bufs=2, xt_bufs=2):
    ablate = set(ablate.split(",")) if ablate else set()
    assert DC == H * depth and DC % P == 0 and DIN % P == 0 and S % SQT == 0
    NKT = S // P          # key chunks of 128
    NDIN = DIN // P       # input-dim k-tiles
    NDO = DC // P         # d_core blocks
    NSQT = S // SQT       # attention q tiles
    NSQC = SQT // P       # 128-query chunks per sqt
    scale = 1.0 / float(depth) ** 0.5

    nc = bacc.Bacc("TRN2", target_bir_lowering=False, debug=False,
                   num_devices=num_devices)
    xq = nc.dram_tensor("xq", [S, DIN], BF16, kind="ExternalInput")
    xk = nc.dram_tensor("xk", [S, DIN], BF16, kind="ExternalInput")
    xv = nc.dram_tensor("xv", [S, DIN], BF16, kind="ExternalInput")
    wq = nc.dram_tensor("wq", [DIN, DC], BF16, kind="ExternalInput")
    wk = nc.dram_tensor("wk", [DIN, DC], BF16, kind="ExternalInput")
    wv = nc.dram_tensor("wv", [DIN, DC], BF16, kind="ExternalInput")
    wo = nc.dram_tensor("wo", [DC, DOUT], BF16, kind="ExternalInput")
    bq = nc.dram_tensor("bq", [DC], F32, kind="ExternalInput")
    bk = nc.dram_tensor("bk", [DC], F32, kind="ExternalInput")
    bv = nc.dram_tensor("bv", [DC], F32, kind="ExternalInput")
    out = nc.dram_tensor("out", [S, DOUT], F32, kind="ExternalOutput")

    with TileContext(nc) as tc, ExitStack() as ctx:
        const = ctx.enter_context(tc.tile_pool(name="const", bufs=1))
        wpool = ctx.enter_context(tc.tile_pool(name="wpool", bufs=1))
        kvpool = ctx.enter_context(tc.tile_pool(name="kv", bufs=1))
        xtkv = ctx.enter_context(tc.tile_pool(name="xtkv", bufs=xt_bufs))
        xtq = ctx.enter_context(tc.tile_pool(name="xtq", bufs=xt_bufs))
        qpool = ctx.enter_context(tc.tile_pool(name="qp", bufs=q_bufs))
        expool = ctx.enter_context(tc.tile_pool(name="ex", bufs=ex_bufs))
        opool = ctx.enter_context(tc.tile_pool(name="op", bufs=2))
        misc = ctx.enter_context(tc.tile_pool(name="misc", bufs=2))
        ps_st = ctx.enter_context(tc.tile_pool(name="ps_st", bufs=st_bufs,
                                               space="PSUM"))
        ps_ot = ctx.enter_context(tc.tile_pool(name="ps_ot", bufs=1,
                                               space="PSUM"))
        ps_gen = ctx.enter_context(tc.tile_pool(name="ps_gen", bufs=2,
                                                space="PSUM"))

        ident = const.tile([P, P], BF16)
        make_identity(nc, ident)

        # ---- weights: direct bf16 DMA loads, no staging ----
        def load_weight(dram, kdim, ndim, name):
            w = wpool.tile([P, kdim // P, ndim], BF16, name=name)
            nc.sync.dma_start(
                w[:], dram[:, :].rearrange("(o p) n -> p o n", p=P))
            return w

        wq_sb = load_weight(wq, DIN, DC, "wq_sb")
        wk_sb = load_weight(wk, DIN, DC, "wk_sb")
        wv_sb = load_weight(wv, DIN, DC, "wv_sb")
        wo_sb = load_weight(wo, DC, DOUT, "wo_sb")

        bq_sb = const.tile([P, NDO], F32)
        nc.sync.dma_start(bq_sb[:], bq[:].rearrange("(o p) -> p o", p=P))
        bk_sb = const.tile([P, NDO], F32)
        nc.sync.dma_start(bk_sb[:], bk[:].rearrange("(o p) -> p o", p=P))
        bv_st = const.tile([1, DC], F32)
        nc.sync.dma_start(bv_st[0:1, :], bv[:][None, :])
        bv_bc = const.tile([P, DC], F32)
        nc.gpsimd.partition_broadcast(bv_bc[:], bv_st[0:1, :])

        KT = kvpool.tile([P, NDO, S], F32R)
        V = kvpool.tile([P, NKT, H, depth + 1], BF16)
        nc.vector.memset(V[:, :, :, depth:depth + 1], 1.0)

        # ---- K projection: KT[d_core, s] via xbar-transposed loads ----
        for st_i in range(S // 512):
            xt = xtkv.tile([P, NDIN, 512], BF16, tag="xt", name="xtk")
            nc.sync.dma_start_transpose(
                xt[:], xk[st_i * 512:(st_i + 1) * 512, :])
            for do in range(NDO):
                ps = ps_gen.tile([P, 512], F32, tag="gen", name="psk")
                for kt in range(NDIN):
                    nc.tensor.matmul(
                        ps[:], wk_sb[:, kt, do * P:(do + 1) * P], xt[:, kt, :],
                        start=(kt == 0), stop=(kt == NDIN - 1))
                nc.scalar.activation(
                    KT[:, do, st_i * 512:(st_i + 1) * 512], ps[:], IDENT,
                    bias=bk_sb[:, do:do + 1])

        # ---- V projection: natural layout, bf16, ones column appended ----
        for st_i in range(S // 512):
            xt = xtkv.tile([P, NDIN, 512], BF16, tag="xt", name="xtv")
            nc.sync.dma_start_transpose(
                xt[:], xv[st_i * 512:(st_i + 1) * 512, :])
            for sc in range(4):  # 128-row chunks
                ps = ps_gen.tile([P, 512], F32, tag="gen", name="psv")
                for kt in range(NDIN):
                    nc.tensor.matmul(
                        ps[:], xt[:, kt, sc * P:(sc + 1) * P], wv_sb[:, kt, :],
                        start=(kt == 0), stop=(kt == NDIN - 1))
                chunk = st_i * 4 + sc
                nc.vector.tensor_tensor(
                    V[:, chunk, :, 0:depth],
                    ps[:].rearrange("p (h d) -> p h d", h=H),
                    bv_bc[:].rearrange("p (h d) -> p h d", h=H),
                    mybir.AluOpType.add)

        # ---- Q projection (pipelined with attention) ----
        def qproj(sqt):
            xt = xtq.tile([P, NDIN, SQT], BF16, tag="xt", name="xtq")
            nc.sync.dma_start_transpose(
                xt[:], xq[sqt * SQT:(sqt + 1) * SQT, :])
            QT = qpool.tile([P, NDO, SQT], F32R, tag="qt", name="qt")
            for do in range(NDO):
                ps = ps_gen.tile([P, 512], F32, tag="gen", name="psq")
                for kt in range(NDIN):
                    nc.tensor.matmul(
                        ps[:], wq_sb[:, kt, do * P:(do + 1) * P], xt[:, kt, :],
                        start=(kt == 0), stop=(kt == NDIN - 1))
                nc.vector.tensor_scalar_add(QT[:, do, :], ps[:],
                                            bq_sb[:, do:do + 1])
            return QT

        NSQT_EFF = 0 if "kvonly" in ablate else NSQT
        QT_next = qproj(0) if NSQT_EFF else None
        for sqt in range(NSQT_EFF):
            QT = QT_next
            if sqt + 1 < NSQT_EFF:
                QT_next = qproj(sqt + 1)

            OTn = opool.tile([P, NDO, SQT], BF16, tag="otn", name="otn")
            for hp in range(H // 2):
                heads = (2 * hp, 2 * hp + 1)
                # padded to a full bank: [128, NSQC, 128] f32 = 2KB/partition
                ot_ps = [ps_ot.tile([P, NSQC, P], F32, name=f"ot{i}")
                         for i in range(2)]
                ex_tiles = {}
                for kt in range(NKT):
                    st = ps_st.tile([P, 2, 512], F32, name="st")
                    for hi, h in enumerate(heads):
                        p0 = (h % 2) * 64
                        nc.tensor.matmul(
                            st[:, hi, :],
                            KT[p0:p0 + 64, hp, kt * P:(kt + 1) * P],
                            QT[p0:p0 + 64, hp, :],
                            start=True, stop=True)
                    ex = expool.tile([P, 2, 512], BF16, tag="ex", name="ex")
                    nc.scalar.activation(ex[:], st[:], EXP, scale=scale)
                    for hi, h in enumerate(heads):
                        for qc in range(NSQC):
                            nc.tensor.matmul(
                                ot_ps[hi][:, qc, 0:depth + 1],
                                ex[:, hi, qc * P:(qc + 1) * P],
                                V[:, kt, h, :],
                                start=(kt == 0), stop=(kt == NKT - 1))
                # normalize (free-dim) and transpose back to d_core-major
                O_sb = opool.tile([P, 2, NSQC, depth], BF16, tag="osb",
                                  name="osb")
                for hi, h in enumerate(heads):
                    rec = misc.tile([P, NSQC, 1], F32, tag="rec", name="rec")
                    nc.vector.reciprocal(rec[:],
                                         ot_ps[hi][:, :, depth:depth + 1])
                    for qc in range(NSQC):
                        nc.vector.tensor_scalar_mul(
                            O_sb[:, hi, qc, :], ot_ps[hi][:, qc, 0:depth],
                            rec[:, qc, :])
                for qc in range(NSQC):
                    tp = ps_gen.tile([P, P], BF16, tag="gen", name="tp")
                    nc.tensor.transpose(tp[:], O_sb[:, :, qc, :], ident[:])
                    nc.vector.tensor_copy(
                        OTn[:, hp, qc * P:(qc + 1) * P], tp[:])

            # out projection for this sqt
            for do in range(DOUT // 512):
                for sc in range(NSQC):
                    ps = ps_gen.tile([P, 512], F32, tag="gen", name="pso")
                    for hh in range(NDO):
                        nc.tensor.matmul(
                            ps[:], OTn[:, hh, sc * P:(sc + 1) * P],
                            wo_sb[:, hh, do * 512:(do + 1) * 512],
                            start=(hh == 0), stop=(hh == NDO - 1))
                    osb = misc.tile([P, 512], F32, tag="osb2", name="osb2")
                    nc.vector.tensor_copy(osb[:], ps[:])
                    r0 = sqt * SQT + sc * P
                    nc.sync.dma_start(out[r0:r0 + P, do * 512:(do + 1) * 512],
                                      osb[:])

    nc.compile()
    return nc


# ---------------------------------------------------------------------------
# Host-side wrapper: shard across 8 NeuronCores, run SPMD, gather.
# Core c handles batch b = c // 2 and head-group g = c % 2 (8 of 16 heads,
# i.e. columns [g*512, (g+1)*512) of Wq/Wk/Wv and rows of Wo).
# ---------------------------------------------------------------------------

import numpy as np
import ml_dtypes

from concourse.bass_utils import run_bass_kernel_spmd

_NC = None
_BF16 = ml_dtypes.bfloat16


def _get_nc():
    global _NC
    if _NC is None:
        _NC = build_mha_core(S=2048, DIN=1024, DC=512, DOUT=1024, H=8,
                             depth=64, num_devices=8)
    return _NC


def _in_maps(q, k, v, Wq, bq, Wk, bk, Wv, bv, Wo, bo):
    f32 = np.float32
    maps = []
    qb = [np.ascontiguousarray(np.asarray(q[b], dtype=f32).astype(_BF16))
          for b in range(4)]
    kb = [np.ascontiguousarray(np.asarray(k[b], dtype=f32).astype(_BF16))
          for b in range(4)]
    vb = [np.ascontiguousarray(np.asarray(v[b], dtype=f32).astype(_BF16))
          for b in range(4)]
    Wq = np.asarray(Wq, dtype=f32)
    Wk = np.asarray(Wk, dtype=f32)
    Wv = np.asarray(Wv, dtype=f32)
    Wo = np.asarray(Wo, dtype=f32)
    for c in range(8):
        b, g = c // 2, c % 2
        sl = slice(g * 512, (g + 1) * 512)
        maps.append({
            "xq": qb[b],
            "xk": kb[b],
            "xv": vb[b],
            "wq": np.ascontiguousarray(Wq[:, sl].astype(_BF16)),
            "wk": np.ascontiguousarray(Wk[:, sl].astype(_BF16)),
            "wv": np.ascontiguousarray(Wv[:, sl].astype(_BF16)),
            "wo": np.ascontiguousarray(Wo[sl, :].astype(_BF16)),
            "bq": np.ascontiguousarray(bq[sl], dtype=f32),
            "bk": np.ascontiguousarray(bk[sl], dtype=f32),
            "bv": np.ascontiguousarray(bv[sl], dtype=f32),
        })
    return maps


def _gather(results, bo):
    out = np.empty((4, 2048, 1024), dtype=np.float32)
    bo32 = np.asarray(bo, dtype=np.float32)
    for b in range(4):
        out[b] = results[2 * b]["out"] + results[2 * b + 1]["out"] + bo32
    return out


def kernel(q, k, v, Wq, bq, Wk, bk, Wv, bv, Wo, bo, _trace=False):
    nc = _get_nc()
    res = run_bass_kernel_spmd(
        nc, _in_maps(q, k, v, Wq, bq, Wk, bk, Wv, bv, Wo, bo),
        core_ids=list(range(8)), trace=_trace)
    out = _gather(res.results, bo)
    if _trace:
        kernel.last_results = res
    return out


# revision 4
# speedup vs baseline: 1.2504x; 1.0010x over previous
"""Bass/Tile multi-head attention kernel builder for TRN2 (v2).

Per-core problem (core c handles batch b=c//2, head-group g=c%2):
  inputs:  xq, xk, xv [S, DIN] bf16     (batch b slices of q/k/v, host-cast)
           wq, wk, wv [DIN, DC] bf16    (column slice for this head group)
           wo [DC, DOUT] bf16           (row slice)
           bq, bk, bv [DC] f32
  output:  out [S, DOUT] f32  partial:  host sums the two head-group partials
           per batch and adds bo.

Math (per head h of H local heads, depth=64):
  xt   = X^T via DMA-xbar transpose loads          [DIN(p-major blocks), S]
  QT   = (wq_blk.T @ xt) + bq                       [DC, S]  f32r
  KT   = (wk_blk.T @ xt) + bk                       [DC, S]  f32r
  V    = (xt_chunk.T @ wv) + bv (+ ones col)        [S, DC(+1/head)] bf16
  ST   = KT_h.T @ QT_h   (64-partition contraction) [keys, q] per head
  E    = exp(ST * 1/sqrt(depth))  -> bf16           (logits O(10), no max-sub)
  OT   = E_chunk.T @ V_aug_h  (transposed-AV)       [q, depth+1] accum over keys
  O    = OT[:, :depth] / OT[:, depth]  -> bf16      (free-dim normalize)
  OTn  = O^T per head pair (PE transpose)           [DC, S] bf16
  out  = OTn.T @ wo                                 [S, DOUT] f32

Layouts (P=128 partitions):
  QT/KT: [128, DC//128, S]  d_core = blk*128 + p (head h -> blk h//2,
         partitions (h%2)*64 ..); scores contract over 64 partitions.
  V:     [128, S//128, H, 65]  key = chunk*128 + p; col 64 = 1.0
  OTn:   [128, DC//128, SQT]  same d_core layout -> out-proj lhsT with K=128
"""

from contextlib import ExitStack

import concourse.mybir as mybir
from concourse import bacc
from concourse.masks import make_identity
from concourse.tile import TileContext

F32 = mybir.dt.float32
F32R = mybir.dt.float32r
BF16 = mybir.dt.bfloat16
P = 128
EXP = mybir.ActivationFunctionType.Exp
IDENT = mybir.ActivationFunctionType.Identity


def build_mha_core(S=2048, DIN=1024, DC=512, DOUT=1024, H=8, depth=64,
                   SQT=512, num_devices=1, ablate="", q_bufs=2, ex_bufs=3,
                   st_bufs=2, xt_bufs=2):
    ablate = set(ablate.split(",")) if ablate else set()
    assert DC == H * depth and DC % P == 0 and DIN % P == 0 and S % SQT == 0
    NKT = S // P          # key chunks of 128
    NDIN = DIN // P       # input-dim k-tiles
    NDO = DC // P         # d_core blocks
    NSQT = S // SQT       # attention q tiles
    NSQC = SQT // P       # 128-query chunks per sqt
    scale = 1.0 / float(depth) ** 0.5

    nc = bacc.Bacc("TRN2", target_bir_lowering=False, debug=False,
                   num_devices=num_devices)
    xq = nc.dram_tensor("xq", [S, DIN], BF16, kind="ExternalInput")
    xk = nc.dram_tensor("xk", [S, DIN], BF16, kind="ExternalInput")
    xv = nc.dram_tensor("xv", [S, DIN], BF16, kind="ExternalInput")
    wq = nc.dram_tensor("wq", [DIN, DC], BF16, kind="ExternalInput")
    wk = nc.dram_tensor("wk", [DIN, DC], BF16, kind="ExternalInput")
    wv = nc.dram_tensor("wv", [DIN, DC], BF16, kind="ExternalInput")
    wo = nc.dram_tensor("wo", [DC, DOUT], BF16, kind="ExternalInput")
    bq = nc.dram_tensor("bq", [DC], F32, kind="ExternalInput")
    bk = nc.dram_tensor("bk", [DC], F32, kind="ExternalInput")
    bv = nc.dram_tensor("bv", [DC], F32, kind="ExternalInput")
    out = nc.dram_tensor("out", [S, DOUT], F32, kind="ExternalOutput")

    with TileContext(nc) as tc, ExitStack() as ctx:
        const = ctx.enter_context(tc.tile_pool(name="const", bufs=1))
        wpool = ctx.enter_context(tc.tile_pool(name="wpool", bufs=1))
        kvpool = ctx.enter_context(tc.tile_pool(name="kv", bufs=1))
        xtkv = ctx.enter_context(tc.tile_pool(name="xtkv", bufs=xt_bufs))
        xtq = ctx.enter_context(tc.tile_pool(name="xtq", bufs=xt_bufs))
        qpool = ctx.enter_context(tc.tile_pool(name="qp", bufs=q_bufs))
        expool = ctx.enter_context(tc.tile_pool(name="ex", bufs=ex_bufs))
        opool = ctx.enter_context(tc.tile_pool(name="op", bufs=2))
        misc = ctx.enter_context(tc.tile_pool(name="misc", bufs=2))
        ps_st = ctx.enter_context(tc.tile_pool(name="ps_st", bufs=st_bufs,
                                               space="PSUM"))
        ps_ot = ctx.enter_context(tc.tile_pool(name="ps_ot", bufs=1,
                                               space="PSUM"))
        ps_gen = ctx.enter_context(tc.tile_pool(name="ps_gen", bufs=2,
                                                space="PSUM"))

        ident = const.tile([P, P], BF16)
        make_identity(nc, ident)

        # ---- weights: direct bf16 DMA loads, no staging ----
        def load_weight(dram, kdim, ndim, name):
            w = wpool.tile([P, kdim // P, ndim], BF16, name=name)
            nc.sync.dma_start(
                w[:], dram[:, :].rearrange("(o p) n -> p o n", p=P))
            return w

        wq_sb = load_weight(wq, DIN, DC, "wq_sb")
        wk_sb = load_weight(wk, DIN, DC, "wk_sb")
        wv_sb = load_weight(wv, DIN, DC, "wv_sb")
        wo_sb = load_weight(wo, DC, DOUT, "wo_sb")

        bq_sb = const.tile([P, NDO], F32)
        nc.sync.dma_start(bq_sb[:], bq[:].rearrange("(o p) -> p o", p=P))
        bk_sb = const.tile([P, NDO], F32)
        nc.sync.dma_start(bk_sb[:], bk[:].rearrange("(o p) -> p o", p=P))
        bv_st = const.tile([1, DC], F32)
        nc.sync.dma_start(bv_st[0:1, :], bv[:][None, :])
        bv_bc = const.tile([P, DC], F32)
        nc.gpsimd.partition_broadcast(bv_bc[:], bv_st[0:1, :])

        KT = kvpool.tile([P, NDO, S], F32R)
        V = kvpool.tile([P, NKT, H, depth + 1], BF16)
        nc.vector.memset(V[:, :, :, depth:depth + 1], 1.0)

        # ---- K projection: KT[d_core, s] via xbar-transposed loads ----
        for st_i in range(S // 512):
            xt = xtkv.tile([P, NDIN, 512], BF16, tag="xt", name="xtk")
            nc.sync.dma_start_transpose(
                xt[:], xk[st_i * 512:(st_i + 1) * 512, :])
            for do in range(NDO):
                ps = ps_gen.tile([P, 512], F32, tag="gen", name="psk")
                for kt in range(NDIN):
                    nc.tensor.matmul(
                        ps[:], wk_sb[:, kt, do * P:(do + 1) * P], xt[:, kt, :],
                        start=(kt == 0), stop=(kt == NDIN - 1))
                nc.vector.tensor_scalar_add(
                    KT[:, do, st_i * 512:(st_i + 1) * 512], ps[:],
                    bk_sb[:, do:do + 1])

        # ---- V projection: natural layout, bf16, ones column appended ----
        for st_i in range(S // 512):
            xt = xtkv.tile([P, NDIN, 512], BF16, tag="xt", name="xtv")
            nc.sync.dma_start_transpose(
                xt[:], xv[st_i * 512:(st_i + 1) * 512, :])
            for sc in range(4):  # 128-row chunks
                ps = ps_gen.tile([P, 512], F32, tag="gen", name="psv")
                for kt in range(NDIN):
                    nc.tensor.matmul(
                        ps[:], xt[:, kt, sc * P:(sc + 1) * P], wv_sb[:, kt, :],
                        start=(kt == 0), stop=(kt == NDIN - 1))
                chunk = st_i * 4 + sc
                nc.vector.tensor_tensor(
                    V[:, chunk, :, 0:depth],
                    ps[:].rearrange("p (h d) -> p h d", h=H),
                    bv_bc[:].rearrange("p (h d) -> p h d", h=H),
                    mybir.AluOpType.add)

        # ---- Q projection (pipelined with attention) ----
        def qproj(sqt):
            xt = xtq.tile([P, NDIN, SQT], BF16, tag="xt", name="xtq")
            nc.sync.dma_start_transpose(
                xt[:], xq[sqt * SQT:(sqt + 1) * SQT, :])
            QT = qpool.tile([P, NDO, SQT], F32R, tag="qt", name="qt")
            for do in range(NDO):
                ps = ps_gen.tile([P, 512], F32, tag="gen", name="psq")
                for kt in range(NDIN):
                    nc.tensor.matmul(
                        ps[:], wq_sb[:, kt, do * P:(do + 1) * P], xt[:, kt, :],
                        start=(kt == 0), stop=(kt == NDIN - 1))
                nc.vector.tensor_scalar_add(QT[:, do, :], ps[:],
                                            bq_sb[:, do:do + 1])
            return QT

        NSQT_EFF = 0 if "kvonly" in ablate else NSQT
        QT_next = qproj(0) if NSQT_EFF else None
        for sqt in range(NSQT_EFF):
            QT = QT_next
            if sqt + 1 < NSQT_EFF:
                QT_next = qproj(sqt + 1)

            OTn = opool.tile([P, NDO, SQT], BF16, tag="otn", name="otn")
            for hp in range(H // 2):
                heads = (2 * hp, 2 * hp + 1)
                # padded to a full bank: [128, NSQC, 128] f32 = 2KB/partition
                ot_ps = [ps_ot.tile([P, NSQC, P], F32, name=f"ot{i}")
                         for i in range(2)]

                def scores_exp(kt):
                    st = ps_st.tile([P, 2, 512], F32, name="st")
                    for hi, h in enumerate(heads):
                        p0 = (h % 2) * 64
                        nc.tensor.matmul(
                            st[:, hi, :],
                            KT[p0:p0 + 64, hp, kt * P:(kt + 1) * P],
                            QT[p0:p0 + 64, hp, :],
                            start=True, stop=True)
                    ex = expool.tile([P, 2, 512], BF16, tag="ex", name="ex")
                    nc.scalar.activation(ex[:], st[:], EXP, scale=scale)
                    return ex

                def av_t(kt, ex):
                    for hi, h in enumerate(heads):
                        for qc in range(NSQC):
                            nc.tensor.matmul(
                                ot_ps[hi][:, qc, 0:depth + 1],
                                ex[:, hi, qc * P:(qc + 1) * P],
                                V[:, kt, h, :],
                                start=(kt == 0), stop=(kt == NKT - 1))

                # 1-deep software pipeline: AV-T for kt runs while exp(kt+1)
                # is on the scalar engine, so PE never head-of-line blocks.
                ex_prev = scores_exp(0)
                for kt in range(1, NKT):
                    ex = scores_exp(kt)
                    av_t(kt - 1, ex_prev)
                    ex_prev = ex
                av_t(NKT - 1, ex_prev)

                # normalize (free-dim) and transpose back to d_core-major
                O_sb = opool.tile([P, 2, NSQC, depth], BF16, tag="osb",
                                  name="osb")
                for hi, h in enumerate(heads):
                    rec = misc.tile([P, NSQC, 1], F32, tag="rec", name="rec")
                    nc.vector.reciprocal(rec[:],
                                         ot_ps[hi][:, :, depth:depth + 1])
                    nc.vector.tensor_tensor(
                        O_sb[:, hi, :, :], ot_ps[hi][:, :, 0:depth],
                        rec[:].to_broadcast((P, NSQC, depth)),
                        mybir.AluOpType.mult)
                for qc in range(NSQC):
                    tp = ps_gen.tile([P, P], BF16, tag="gen", name="tp")
                    nc.tensor.transpose(tp[:], O_sb[:, :, qc, :], ident[:])
                    nc.vector.tensor_copy(
                        OTn[:, hp, qc * P:(qc + 1) * P], tp[:])

            # out projection for this sqt
            for do in range(DOUT // 512):
                for sc in range(NSQC):
                    ps = ps_gen.tile([P, 512], F32, tag="gen", name="pso")
                    for hh in range(NDO):
                        nc.tensor.matmul(
                            ps[:], OTn[:, hh, sc * P:(sc + 1) * P],
                            wo_sb[:, hh, do * 512:(do + 1) * 512],
                            start=(hh == 0), stop=(hh == NDO - 1))
                    osb = misc.tile([P, 512], F32, tag="osb2", name="osb2")
                    nc.vector.tensor_copy(osb[:], ps[:])
                    r0 = sqt * SQT + sc * P
                    nc.sync.dma_start(out[r0:r0 + P, do * 512:(do + 1) * 512],
                                      osb[:])

    nc.compile()
    return nc


# ---------------------------------------------------------------------------
# Host-side wrapper: shard across 8 NeuronCores, run SPMD, gather.
# Core c handles batch b = c // 2 and head-group g = c % 2 (8 of 16 heads,
# i.e. columns [g*512, (g+1)*512) of Wq/Wk/Wv and rows of Wo).
# ---------------------------------------------------------------------------

import numpy as np
import ml_dtypes

from concourse.bass_utils import run_bass_kernel_spmd

_NC = None
_BF16 = ml_dtypes.bfloat16


def _get_nc():
    global _NC
    if _NC is None:
        _NC = build_mha_core(S=2048, DIN=1024, DC=512, DOUT=1024, H=8,
                             depth=64, num_devices=8)
    return _NC


def _in_maps(q, k, v, Wq, bq, Wk, bk, Wv, bv, Wo, bo):
    f32 = np.float32
    maps = []
    qb = [np.ascontiguousarray(np.asarray(q[b], dtype=f32).astype(_BF16))
          for b in range(4)]
    kb = [np.ascontiguousarray(np.asarray(k[b], dtype=f32).astype(_BF16))
          for b in range(4)]
    vb = [np.ascontiguousarray(np.asarray(v[b], dtype=f32).astype(_BF16))
          for b in range(4)]
    Wq = np.asarray(Wq, dtype=f32)
    Wk = np.asarray(Wk, dtype=f32)
    Wv = np.asarray(Wv, dtype=f32)
    Wo = np.asarray(Wo, dtype=f32)
    for c in range(8):
        b, g = c // 2, c % 2
        sl = slice(g * 512, (g + 1) * 512)
        maps.append({
            "xq": qb[b],
            "xk": kb[b],
            "xv": vb[b],
            "wq": np.ascontiguousarray(Wq[:, sl].astype(_BF16)),
            "wk": np.ascontiguousarray(Wk[:, sl].astype(_BF16)),
            "wv": np.ascontiguousarray(Wv[:, sl].astype(_BF16)),
            "wo": np.ascontiguousarray(Wo[sl, :].astype(_BF16)),
            "bq": np.ascontiguousarray(bq[sl], dtype=f32),
            "bk": np.ascontiguousarray(bk[sl], dtype=f32),
            "bv": np.ascontiguousarray(bv[sl], dtype=f32),
        })
    return maps


def _gather(results, bo):
    out = np.empty((4, 2048, 1024), dtype=np.float32)
    bo32 = np.asarray(bo, dtype=np.float32)
    for b in range(4):
        out[b] = results[2 * b]["out"] + results[2 * b + 1]["out"] + bo32
    return out


def kernel(q, k, v, Wq, bq, Wk, bk, Wv, bv, Wo, bo, _trace=False):
    nc = _get_nc()
    res = run_bass_kernel_spmd(
        nc, _in_maps(q, k, v, Wq, bq, Wk, bk, Wv, bv, Wo, bo),
        core_ids=list(range(8)), trace=_trace)
    out = _gather(res.results, bo)
    if _trace:
        kernel.last_results = res
    return out


# revision 5
# speedup vs baseline: 1.3617x; 1.0891x over previous
"""Bass/Tile multi-head attention kernel builder for TRN2 (v3).

Per-core problem (core c handles batch b=c//2, head-group g=c%2):
  inputs:  xq, xk, xv [S, DIN] bf16     (batch b slices of q/k/v, host-cast)
           wq, wk, wv [DIN, DC] bf16    (column slice for this head group)
           wo [DC, DOUT] bf16           (row slice)
           bq, bk, bv [DC] f32
  output:  out [S, DOUT] f32  partial:  host sums the two head-group partials
           per batch and adds bo.

Math (per head h of H local heads, depth=64):
  xt   = X^T via DMA-xbar transpose loads          [DIN(p-major blocks), S]
  QT   = (wq_blk.T @ xt) + bq                       [DC, S]  f32r
  KT   = (wk_blk.T @ xt) + bk                       [DC, S]  f32r
  V    = (xt_chunk.T @ wv) + bv (+ ones col)        [S, DC(+1/head)] bf16
  ST   = KT_h.T @ QT_h   (64-partition contraction) [keys, q] per head
  E    = exp(ST * 1/sqrt(depth))  -> bf16           (logits O(10), no max-sub)
  OT   = E_chunk.T @ V_aug_h  (transposed-AV)       [q, depth+1] accum over keys
  O    = OT[:, :depth] / OT[:, depth]  -> bf16      (free-dim normalize)
  OTn  = O^T per head pair (PE transpose)           [DC, S] bf16
  out  = OTn.T @ wo                                 [S, DOUT] f32

The emission order is a hand-rolled software pipeline: the scalar engine
(exp over all S^2 logits) is the throughput floor, so score/exp work is
interleaved into the K/V projection phases and the per-sqt out/Q
projections are spread as PE filler inside the attention kt loop, keeping
both PE and ACT continuously fed.
"""

from collections import deque
from contextlib import ExitStack

import concourse.mybir as mybir
from concourse import bacc
from concourse.masks import make_identity
from concourse.tile import TileContext

F32 = mybir.dt.float32
F32R = mybir.dt.float32r
BF16 = mybir.dt.bfloat16
P = 128
EXP = mybir.ActivationFunctionType.Exp


def build_mha_core(S=2048, DIN=1024, DC=512, DOUT=1024, H=8, depth=64,
                   SQT=512, num_devices=1, ablate="", q_bufs=2, ex_bufs=20,
                   st_bufs=2, xt_bufs=2):
    ablate = set(ablate.split(",")) if ablate else set()
    assert DC == H * depth and DC % P == 0 and DIN % P == 0 and S % SQT == 0
    NKT = S // P          # key chunks of 128
    NDIN = DIN // P       # input-dim k-tiles
    NDO = DC // P         # d_core blocks
    NSQT = S // SQT       # attention q tiles
    NSQC = SQT // P       # 128-query chunks per sqt
    scale = 1.0 / float(depth) ** 0.5

    nc = bacc.Bacc("TRN2", target_bir_lowering=False, debug=False,
                   num_devices=num_devices)
    xq = nc.dram_tensor("xq", [S, DIN], BF16, kind="ExternalInput")
    xk = nc.dram_tensor("xk", [S, DIN], BF16, kind="ExternalInput")
    xv = nc.dram_tensor("xv", [S, DIN], BF16, kind="ExternalInput")
    wq = nc.dram_tensor("wq", [DIN, DC], BF16, kind="ExternalInput")
    wk = nc.dram_tensor("wk", [DIN, DC], BF16, kind="ExternalInput")
    wv = nc.dram_tensor("wv", [DIN, DC], BF16, kind="ExternalInput")
    wo = nc.dram_tensor("wo", [DC, DOUT], BF16, kind="ExternalInput")
    bq = nc.dram_tensor("bq", [DC], F32, kind="ExternalInput")
    bk = nc.dram_tensor("bk", [DC], F32, kind="ExternalInput")
    bv = nc.dram_tensor("bv", [DC], F32, kind="ExternalInput")
    out = nc.dram_tensor("out", [S, DOUT], F32, kind="ExternalOutput")

    with TileContext(nc) as tc, ExitStack() as ctx:
        const = ctx.enter_context(tc.tile_pool(name="const", bufs=1))
        wpool = ctx.enter_context(tc.tile_pool(name="wpool", bufs=1))
        kvpool = ctx.enter_context(tc.tile_pool(name="kv", bufs=1))
        xtkv = ctx.enter_context(tc.tile_pool(name="xtkv", bufs=xt_bufs))
        xtq = ctx.enter_context(tc.tile_pool(name="xtq", bufs=xt_bufs))
        qpool = ctx.enter_context(tc.tile_pool(name="qp", bufs=q_bufs))
        expool = ctx.enter_context(tc.tile_pool(name="ex", bufs=ex_bufs))
        opool = ctx.enter_context(tc.tile_pool(name="op", bufs=2))
        misc = ctx.enter_context(tc.tile_pool(name="misc", bufs=2))
        ps_st = ctx.enter_context(tc.tile_pool(name="ps_st", bufs=st_bufs,
                                               space="PSUM"))
        ps_ot = ctx.enter_context(tc.tile_pool(name="ps_ot", bufs=1,
                                               space="PSUM"))
        ps_gen = ctx.enter_context(tc.tile_pool(name="ps_gen", bufs=2,
                                                space="PSUM"))

        ident = const.tile([P, P], BF16)
        make_identity(nc, ident)

        # ---- weights: direct bf16 DMA loads, no staging ----
        def load_weight(dram, kdim, ndim, name):
            w = wpool.tile([P, kdim // P, ndim], BF16, name=name)
            nc.sync.dma_start(
                w[:], dram[:, :].rearrange("(o p) n -> p o n", p=P))
            return w

        wq_sb = load_weight(wq, DIN, DC, "wq_sb")
        bq_sb = const.tile([P, NDO], F32)
        nc.sync.dma_start(bq_sb[:], bq[:].rearrange("(o p) -> p o", p=P))
        wk_sb = load_weight(wk, DIN, DC, "wk_sb")
        bk_sb = const.tile([P, NDO], F32)
        nc.sync.dma_start(bk_sb[:], bk[:].rearrange("(o p) -> p o", p=P))
        wv_sb = load_weight(wv, DIN, DC, "wv_sb")
        wo_sb = load_weight(wo, DC, DOUT, "wo_sb")
        bv_st = const.tile([1, DC], F32)
        nc.sync.dma_start(bv_st[0:1, :], bv[:][None, :])
        bv_bc = const.tile([P, DC], F32)
        nc.gpsimd.partition_broadcast(bv_bc[:], bv_st[0:1, :])

        KT = kvpool.tile([P, NDO, S], F32R)
        V = kvpool.tile([P, NKT, H, depth + 1], BF16)
        nc.vector.memset(V[:, :, :, depth:depth + 1], 1.0)

        # ---------------- emitters ----------------
        def kproj_chunk(st_i):
            xt = xtkv.tile([P, NDIN, 512], BF16, tag="xt", name="xtk")
            nc.sync.dma_start_transpose(
                xt[:], xk[st_i * 512:(st_i + 1) * 512, :])
            for do in range(NDO):
                ps = ps_gen.tile([P, 512], F32, tag="gen", name="psk")
                for kt in range(NDIN):
                    nc.tensor.matmul(
                        ps[:], wk_sb[:, kt, do * P:(do + 1) * P], xt[:, kt, :],
                        start=(kt == 0), stop=(kt == NDIN - 1))
                nc.vector.tensor_scalar_add(
                    KT[:, do, st_i * 512:(st_i + 1) * 512], ps[:],
                    bk_sb[:, do:do + 1])

        def vproj_chunk(st_i):
            xt = xtkv.tile([P, NDIN, 512], BF16, tag="xt", name="xtv")
            nc.sync.dma_start_transpose(
                xt[:], xv[st_i * 512:(st_i + 1) * 512, :])
            for sc in range(4):
                ps = ps_gen.tile([P, 512], F32, tag="gen", name="psv")
                for kt in range(NDIN):
                    nc.tensor.matmul(
                        ps[:], xt[:, kt, sc * P:(sc + 1) * P], wv_sb[:, kt, :],
                        start=(kt == 0), stop=(kt == NDIN - 1))
                chunk = st_i * 4 + sc
                nc.vector.tensor_tensor(
                    V[:, chunk, :, 0:depth],
                    ps[:].rearrange("p (h d) -> p h d", h=H),
                    bv_bc[:].rearrange("p (h d) -> p h d", h=H),
                    mybir.AluOpType.add)

        QTs = {}

        def qproj_load(sqt):
            xt = xtq.tile([P, NDIN, SQT], BF16, tag="xt", name="xtq")
            nc.sync.dma_start_transpose(
                xt[:], xq[sqt * SQT:(sqt + 1) * SQT, :])
            QTs[sqt] = (qpool.tile([P, NDO, SQT], F32R, tag="qt", name="qt"),
                        xt)

        def qproj_chain(sqt, do):
            QT, xt = QTs[sqt]
            ps = ps_gen.tile([P, 512], F32, tag="gen", name="psq")
            for kt in range(NDIN):
                nc.tensor.matmul(
                    ps[:], wq_sb[:, kt, do * P:(do + 1) * P], xt[:, kt, :],
                    start=(kt == 0), stop=(kt == NDIN - 1))
            nc.vector.tensor_scalar_add(QT[:, do, :], ps[:],
                                        bq_sb[:, do:do + 1])

        ex_map = {}
        ot_map = {}
        OTns = {}

        def scores_exp(sqt, hp, kt):
            QT = QTs[sqt][0]
            st = ps_st.tile([P, 2, 512], F32, name="st")
            for hi, h in enumerate((2 * hp, 2 * hp + 1)):
                p0 = (h % 2) * 64
                nc.tensor.matmul(
                    st[:, hi, :],
                    KT[p0:p0 + 64, hp, kt * P:(kt + 1) * P],
                    QT[p0:p0 + 64, hp, :],
                    start=True, stop=True)
            ex = expool.tile([P, 2, 512], BF16, tag="ex", name="ex")
            nc.scalar.activation(ex[:], st[:], EXP, scale=scale)
            ex_map[(sqt, hp, kt)] = ex

        def av_t(sqt, hp, kt):
            if kt == 0:
                ot_map[(sqt, hp)] = [
                    ps_ot.tile([P, NSQC, P], F32, name=f"ot{i}")
                    for i in range(2)]
            ot_ps = ot_map[(sqt, hp)]
            ex = ex_map.pop((sqt, hp, kt))
            for hi, h in enumerate((2 * hp, 2 * hp + 1)):
                for qc in range(NSQC):
                    nc.tensor.matmul(
                        ot_ps[hi][:, qc, 0:depth + 1],
                        ex[:, hi, qc * P:(qc + 1) * P],
                        V[:, kt, h, :],
                        start=(kt == 0), stop=(kt == NKT - 1))

        def norm_transp(sqt, hp):
            if hp == 0:
                OTns[sqt] = opool.tile([P, NDO, SQT], BF16, tag="otn",
                                       name="otn")
            OTn = OTns[sqt]
            ot_ps = ot_map.pop((sqt, hp))
            O_sb = opool.tile([P, 2, NSQC, depth], BF16, tag="osb",
                              name="osb")
            for hi in range(2):
                rec = misc.tile([P, NSQC, 1], F32, tag="rec", name="rec")
                nc.vector.reciprocal(rec[:],
                                     ot_ps[hi][:, :, depth:depth + 1])
                nc.vector.tensor_tensor(
                    O_sb[:, hi, :, :], ot_ps[hi][:, :, 0:depth],
                    rec[:].to_broadcast((P, NSQC, depth)),
                    mybir.AluOpType.mult)
            for qc in range(NSQC):
                tp = ps_gen.tile([P, P], BF16, tag="gen", name="tp")
                nc.tensor.transpose(tp[:], O_sb[:, :, qc, :], ident[:])
                nc.vector.tensor_copy(
                    OTn[:, hp, qc * P:(qc + 1) * P], tp[:])

        def oproj_chain(sqt, do, sc):
            OTn = OTns[sqt]
            ps = ps_gen.tile([P, 512], F32, tag="gen", name="pso")
            for hh in range(NDO):
                nc.tensor.matmul(
                    ps[:], OTn[:, hh, sc * P:(sc + 1) * P],
                    wo_sb[:, hh, do * 512:(do + 1) * 512],
                    start=(hh == 0), stop=(hh == NDO - 1))
            osb = misc.tile([P, 512], F32, tag="osb2", name="osb2")
            nc.vector.tensor_copy(osb[:], ps[:])
            r0 = sqt * SQT + sc * P
            nc.sync.dma_start(out[r0:r0 + P, do * 512:(do + 1) * 512],
                              osb[:])

        # ---------------- schedule ----------------
        # Phase B: Q-proj(0) + K-proj, with sqt0/hp0 scores+exp fused in.
        qproj_load(0)
        for do in range(NDO):
            qproj_chain(0, do)
        for st_i in range(4):
            kproj_chunk(st_i)
            for kt in range(4 * st_i, 4 * st_i + 4):
                if kt < 12:
                    scores_exp(0, 0, kt)
        # Phase C: V-proj groups; keep feeding exp, start hp0 AV-T lagged
        # one group behind the V chunks it needs.
        for st_i in range(4):
            vproj_chunk(st_i)
            if st_i == 0:
                for kt in range(12, 16):
                    scores_exp(0, 0, kt)
            else:
                for kt in range(4 * st_i - 4, 4 * st_i):
                    scores_exp(0, 1, kt)
                    av_t(0, 0, kt)
        # Phase D: finish sqt0 (hp1..hp3), qproj(1) as filler.
        qproj_load(1)
        filler = deque()
        for do in range(NDO):
            filler.append((qproj_chain, (1, do)))
        for kt in range(12, 16):
            scores_exp(0, 1, kt)
            av_t(0, 0, kt)
        norm_transp(0, 0)
        for kt in range(NKT):
            if kt > 0:
                av_t(0, 1, kt - 1)
            if kt % 4 == 1 and filler:
                f, a = filler.popleft()
                f(*a)
        av_t(0, 1, NKT - 1)
        norm_transp(0, 1)
        for hp in (2, 3):
            for kt in range(NKT):
                scores_exp(0, hp, kt)
                if kt > 0:
                    av_t(0, hp, kt - 1)
                if kt % 4 == 1 and filler:
                    f, a = filler.popleft()
                    f(*a)
            av_t(0, hp, NKT - 1)
            norm_transp(0, hp)

        # Phase E: steady sqt loop; previous sqt's out-proj and next sqt's
        # Q-proj interleaved as PE filler between score/exp pairs.
        for sqt in range(1, NSQT):
            if sqt + 1 < NSQT:
                qproj_load(sqt + 1)
            filler = deque()
            if sqt + 1 < NSQT:
                for do in range(NDO):
                    filler.append((qproj_chain, (sqt + 1, do)))
            for do in range(DOUT // 512):
                for sc in range(NSQC):
                    filler.append((oproj_chain, (sqt - 1, do, sc)))
            for hp in range(H // 2):
                for kt in range(NKT):
                    scores_exp(sqt, hp, kt)
                    if kt > 0:
                        av_t(sqt, hp, kt - 1)
                    if kt % 4 == 1 and filler:
                        f, a = filler.popleft()
                        f(*a)
                av_t(sqt, hp, NKT - 1)
                norm_transp(sqt, hp)
            while filler:
                f, a = filler.popleft()
                f(*a)
        # tail: out-proj of the last sqt
        for do in range(DOUT // 512):
            for sc in range(NSQC):
                oproj_chain(NSQT - 1, do, sc)

    nc.compile()
    return nc


# ---------------------------------------------------------------------------
# Host-side wrapper: shard across 8 NeuronCores, run SPMD, gather.
# Core c handles batch b = c // 2 and head-group g = c % 2 (8 of 16 heads,
# i.e. columns [g*512, (g+1)*512) of Wq/Wk/Wv and rows of Wo).
# ---------------------------------------------------------------------------

import numpy as np
import ml_dtypes

from concourse.bass_utils import run_bass_kernel_spmd

_NC = None
_BF16 = ml_dtypes.bfloat16


def _get_nc():
    global _NC
    if _NC is None:
        _NC = build_mha_core(S=2048, DIN=1024, DC=512, DOUT=1024, H=8,
                             depth=64, num_devices=8)
    return _NC


def _in_maps(q, k, v, Wq, bq, Wk, bk, Wv, bv, Wo, bo):
    f32 = np.float32
    maps = []
    qb = [np.ascontiguousarray(np.asarray(q[b], dtype=f32).astype(_BF16))
          for b in range(4)]
    kb = [np.ascontiguousarray(np.asarray(k[b], dtype=f32).astype(_BF16))
          for b in range(4)]
    vb = [np.ascontiguousarray(np.asarray(v[b], dtype=f32).astype(_BF16))
          for b in range(4)]
    Wq = np.asarray(Wq, dtype=f32)
    Wk = np.asarray(Wk, dtype=f32)
    Wv = np.asarray(Wv, dtype=f32)
    Wo = np.asarray(Wo, dtype=f32)
    for c in range(8):
        b, g = c // 2, c % 2
        sl = slice(g * 512, (g + 1) * 512)
        maps.append({
            "xq": qb[b],
            "xk": kb[b],
            "xv": vb[b],
            "wq": np.ascontiguousarray(Wq[:, sl].astype(_BF16)),
            "wk": np.ascontiguousarray(Wk[:, sl].astype(_BF16)),
            "wv": np.ascontiguousarray(Wv[:, sl].astype(_BF16)),
            "wo": np.ascontiguousarray(Wo[sl, :].astype(_BF16)),
            "bq": np.ascontiguousarray(bq[sl], dtype=f32),
            "bk": np.ascontiguousarray(bk[sl], dtype=f32),
            "bv": np.ascontiguousarray(bv[sl], dtype=f32),
        })
    return maps


def _gather(results, bo):
    out = np.empty((4, 2048, 1024), dtype=np.float32)
    bo32 = np.asarray(bo, dtype=np.float32)
    for b in range(4):
        out[b] = results[2 * b]["out"] + results[2 * b + 1]["out"] + bo32
    return out


def kernel(q, k, v, Wq, bq, Wk, bk, Wv, bv, Wo, bo, _trace=False):
    nc = _get_nc()
    res = run_bass_kernel_spmd(
        nc, _in_maps(q, k, v, Wq, bq, Wk, bk, Wv, bv, Wo, bo),
        core_ids=list(range(8)), trace=_trace)
    out = _gather(res.results, bo)
    if _trace:
        kernel.last_results = res
    return out


# revision 10
# speedup vs baseline: 1.4293x; 1.0496x over previous
"""Bass/Tile multi-head attention kernel builder for TRN2 (v3).

Per-core problem (core c handles batch b=c//2, head-group g=c%2):
  inputs:  xq, xk, xv [S, DIN] bf16     (batch b slices of q/k/v, host-cast)
           wq, wk, wv [DIN, DC] bf16    (column slice for this head group)
           wo [DC, DOUT] bf16           (row slice)
           bq, bk, bv [DC] f32
  output:  out [S, DOUT] f32  partial:  host sums the two head-group partials
           per batch and adds bo.

Math (per head h of H local heads, depth=64):
  xt   = X^T via DMA-xbar transpose loads          [DIN(p-major blocks), S]
  QT   = (wq_blk.T @ xt) + bq                       [DC, S]  f32r
  KT   = (wk_blk.T @ xt) + bk                       [DC, S]  f32r
  V    = (xt_chunk.T @ wv) + bv (+ ones col)        [S, DC(+1/head)] bf16
  ST   = KT_h.T @ QT_h   (64-partition contraction) [keys, q] per head
  E    = exp(ST * 1/sqrt(depth))  -> bf16           (logits O(10), no max-sub)
  OT   = E_chunk.T @ V_aug_h  (transposed-AV)       [q, depth+1] accum over keys
  O    = OT[:, :depth] / OT[:, depth]  -> bf16      (free-dim normalize)
  OTn  = O^T per head pair (PE transpose)           [DC, S] bf16
  out  = OTn.T @ wo                                 [S, DOUT] f32

The emission order is a hand-rolled software pipeline: the scalar engine
(exp over all S^2 logits) is the throughput floor, so score/exp work is
interleaved into the K/V projection phases and the per-sqt out/Q
projections are spread as PE filler inside the attention kt loop, keeping
both PE and ACT continuously fed.
"""

from collections import deque
from contextlib import ExitStack

import concourse.mybir as mybir
from concourse import bacc
from concourse.masks import make_identity
from concourse.tile import TileContext

F32 = mybir.dt.float32
F32R = mybir.dt.float32r
BF16 = mybir.dt.bfloat16
P = 128
EXP = mybir.ActivationFunctionType.Exp


def build_mha_core(S=2048, DIN=1024, DC=512, DOUT=1024, H=8, depth=64,
                   SQT=512, num_devices=1, ablate="", q_bufs=2, ex_bufs=20,
                   st_bufs=2, xt_bufs=2):
    ablate = set(ablate.split(",")) if ablate else set()
    assert DC == H * depth and DC % P == 0 and DIN % P == 0 and S % SQT == 0
    NKT = S // P          # key chunks of 128
    NDIN = DIN // P       # input-dim k-tiles
    NDO = DC // P         # d_core blocks
    NSQT = S // SQT       # attention q tiles
    NSQC = SQT // P       # 128-query chunks per sqt
    scale = 1.0 / float(depth) ** 0.5

    nc = bacc.Bacc("TRN2", target_bir_lowering=False, debug=False,
                   num_devices=num_devices)
    xq = nc.dram_tensor("xq", [S, DIN], BF16, kind="ExternalInput")
    xk = nc.dram_tensor("xk", [S, DIN], BF16, kind="ExternalInput")
    xv = nc.dram_tensor("xv", [S, DIN], BF16, kind="ExternalInput")
    wq = nc.dram_tensor("wq", [DIN, DC], BF16, kind="ExternalInput")
    wk = nc.dram_tensor("wk", [DIN, DC], BF16, kind="ExternalInput")
    wv = nc.dram_tensor("wv", [DIN, DC], BF16, kind="ExternalInput")
    wo = nc.dram_tensor("wo", [DC, DOUT], BF16, kind="ExternalInput")
    bq = nc.dram_tensor("bq", [DC], F32, kind="ExternalInput")
    bk = nc.dram_tensor("bk", [DC], F32, kind="ExternalInput")
    bv = nc.dram_tensor("bv", [DC], F32, kind="ExternalInput")
    out = nc.dram_tensor("out", [S, DOUT], F32, kind="ExternalOutput")

    with TileContext(nc) as tc, ExitStack() as ctx:
        const = ctx.enter_context(tc.tile_pool(name="const", bufs=1))
        wpool = ctx.enter_context(tc.tile_pool(name="wpool", bufs=1))
        kvpool = ctx.enter_context(tc.tile_pool(name="kv", bufs=1))
        xtkv = ctx.enter_context(tc.tile_pool(name="xtkv", bufs=xt_bufs))
        xtq = ctx.enter_context(tc.tile_pool(name="xtq", bufs=xt_bufs))
        qpool = ctx.enter_context(tc.tile_pool(name="qp", bufs=q_bufs))
        expool = ctx.enter_context(tc.tile_pool(name="ex", bufs=ex_bufs))
        opool = ctx.enter_context(tc.tile_pool(name="op", bufs=2))
        misc = ctx.enter_context(tc.tile_pool(name="misc", bufs=2))
        ps_st = ctx.enter_context(tc.tile_pool(name="ps_st", bufs=st_bufs,
                                               space="PSUM"))
        ps_ot = ctx.enter_context(tc.tile_pool(name="ps_ot", bufs=1,
                                               space="PSUM"))
        ps_gen = ctx.enter_context(tc.tile_pool(name="ps_gen", bufs=2,
                                                space="PSUM"))

        ident = const.tile([P, P], BF16)
        make_identity(nc, ident)

        # ---- weights: direct bf16 DMA loads, no staging ----
        def load_weight(dram, kdim, ndim, name):
            w = wpool.tile([P, kdim // P, ndim], BF16, name=name)
            nc.sync.dma_start(
                w[:], dram[:, :].rearrange("(o p) n -> p o n", p=P))
            return w

        KT = kvpool.tile([P, NDO, S], F32R)
        V = kvpool.tile([P, NKT, H, depth + 1], BF16)
        nc.vector.memset(V[:, :, :, depth:depth + 1], 1.0)

        # ---------------- emitters ----------------
        def kproj_chunk(st_i):
            xt = xtkv.tile([P, NDIN, 512], BF16, tag="xt", name="xtk")
            nc.sync.dma_start_transpose(
                xt[:], xk[st_i * 512:(st_i + 1) * 512, :])
            for do in range(NDO):
                ps = ps_gen.tile([P, 512], F32, tag="gen", name="psk")
                for kt in range(NDIN):
                    nc.tensor.matmul(
                        ps[:], wk_sb[:, kt, do * P:(do + 1) * P], xt[:, kt, :],
                        start=(kt == 0), stop=(kt == NDIN - 1))
                nc.vector.tensor_scalar_add(
                    KT[:, do, st_i * 512:(st_i + 1) * 512], ps[:],
                    bk_sb[:, do:do + 1])

        def vproj_chunk(st_i):
            xt = xtkv.tile([P, NDIN, 512], BF16, tag="xt", name="xtv")
            nc.sync.dma_start_transpose(
                xt[:], xv[st_i * 512:(st_i + 1) * 512, :])
            for sc in range(4):
                ps = ps_gen.tile([P, 512], F32, tag="gen", name="psv")
                for kt in range(NDIN):
                    nc.tensor.matmul(
                        ps[:], xt[:, kt, sc * P:(sc + 1) * P], wv_sb[:, kt, :],
                        start=(kt == 0), stop=(kt == NDIN - 1))
                chunk = st_i * 4 + sc
                nc.vector.tensor_tensor(
                    V[:, chunk, :, 0:depth],
                    ps[:].rearrange("p (h d) -> p h d", h=H),
                    bv_bc[:].rearrange("p (h d) -> p h d", h=H),
                    mybir.AluOpType.add)

        QTs = {}

        def qproj_load(sqt):
            xt = xtq.tile([P, NDIN, SQT], BF16, tag="xt", name="xtq")
            nc.sync.dma_start_transpose(
                xt[:], xq[sqt * SQT:(sqt + 1) * SQT, :])
            QTs[sqt] = (qpool.tile([P, NDO, SQT], F32R, tag="qt", name="qt"),
                        xt)

        def qproj_chain(sqt, do):
            QT, xt = QTs[sqt]
            ps = ps_gen.tile([P, 512], F32, tag="gen", name="psq")
            for kt in range(NDIN):
                nc.tensor.matmul(
                    ps[:], wq_sb[:, kt, do * P:(do + 1) * P], xt[:, kt, :],
                    start=(kt == 0), stop=(kt == NDIN - 1))
            nc.vector.tensor_scalar_add(QT[:, do, :], ps[:],
                                        bq_sb[:, do:do + 1])

        ex_map = {}
        ot_map = {}
        OTns = {}

        def scores_exp(sqt, hp, kt):
            QT = QTs[sqt][0]
            st = ps_st.tile([P, 2, 512], F32, name="st")
            for hi, h in enumerate((2 * hp, 2 * hp + 1)):
                p0 = (h % 2) * 64
                nc.tensor.matmul(
                    st[:, hi, :],
                    KT[p0:p0 + 64, hp, kt * P:(kt + 1) * P],
                    QT[p0:p0 + 64, hp, :],
                    start=True, stop=True)
            ex = expool.tile([P, 2, 512], BF16, tag="ex", name="ex")
            nc.scalar.activation(ex[:], st[:], EXP, scale=scale)
            ex_map[(sqt, hp, kt)] = ex

        def av_t(sqt, hp, kt):
            if kt == 0:
                ot_map[(sqt, hp)] = [
                    ps_ot.tile([P, NSQC, P], F32, name=f"ot{i}")
                    for i in range(2)]
            ot_ps = ot_map[(sqt, hp)]
            ex = ex_map.pop((sqt, hp, kt))
            for hi, h in enumerate((2 * hp, 2 * hp + 1)):
                for qc in range(NSQC):
                    # start zeroes the whole 2KB PSUM zero-region (bank), so
                    # only the very first matmul into each head's bank starts
                    # the group; all four qc regions then accumulate onto
                    # zeros.
                    nc.tensor.matmul(
                        ot_ps[hi][:, qc, 0:depth + 1],
                        ex[:, hi, qc * P:(qc + 1) * P],
                        V[:, kt, h, :],
                        start=(kt == 0 and qc == 0),
                        stop=(kt == NKT - 1 and qc == NSQC - 1),
                        skip_group_check=True)

        def norm_transp(sqt, hp, oproj_after_qc=False):
            if hp == 0:
                OTns[sqt] = opool.tile([P, NDO, SQT], BF16, tag="otn",
                                       name="otn")
            OTn = OTns[sqt]
            ot_ps = ot_map.pop((sqt, hp))
            O_sb = opool.tile([P, NSQC, 2, depth], BF16, tag="osb",
                              name="osb")
            for hi in range(2):
                rec = misc.tile([P, NSQC, 1], F32, tag="rec", name="rec")
                nc.vector.reciprocal(rec[:],
                                     ot_ps[hi][:, :, depth:depth + 1])
                nc.vector.tensor_tensor(
                    O_sb[:, :, hi, :], ot_ps[hi][:, :, 0:depth],
                    rec[:].to_broadcast((P, NSQC, depth)),
                    mybir.AluOpType.mult)
            for qc in range(NSQC):
                tp = ps_gen.tile([P, P], BF16, tag="gen", name="tp")
                nc.tensor.transpose(tp[:], O_sb[:, qc, :, :], ident[:])
                nc.vector.tensor_copy(
                    OTn[:, hp, qc * P:(qc + 1) * P], tp[:])
                if oproj_after_qc:
                    for do in range(DOUT // 512):
                        oproj_chain(sqt, do, qc)

        def oproj_chain(sqt, do, sc):
            OTn = OTns[sqt]
            ps = ps_gen.tile([P, 512], F32, tag="gen", name="pso")
            for hh in range(NDO):
                nc.tensor.matmul(
                    ps[:], OTn[:, hh, sc * P:(sc + 1) * P],
                    wo_sb[:, hh, do * 512:(do + 1) * 512],
                    start=(hh == 0), stop=(hh == NDO - 1))
            osb = misc.tile([P, 512], F32, tag="osb2", name="osb2")
            nc.vector.tensor_copy(osb[:], ps[:])
            r0 = sqt * SQT + sc * P
            nc.sync.dma_start(out[r0:r0 + P, do * 512:(do + 1) * 512],
                              osb[:])

        # ---------------- schedule ----------------
        # Loads ordered so the first Q/K projection work is unblocked ASAP.
        qproj_load(0)
        wq_sb = load_weight(wq, DIN, DC, "wq_sb")
        bq_sb = const.tile([P, NDO], F32)
        nc.sync.dma_start(bq_sb[:], bq[:].rearrange("(o p) -> p o", p=P))
        for do in range(NDO):
            qproj_chain(0, do)
        wk_sb = load_weight(wk, DIN, DC, "wk_sb")
        bk_sb = const.tile([P, NDO], F32)
        nc.sync.dma_start(bk_sb[:], bk[:].rearrange("(o p) -> p o", p=P))

        # Phase B: K-proj, with sqt0 hp0 (and first hp1) scores+exp fused in.
        for st_i in range(4):
            kproj_chunk(st_i)
            if st_i == 0:
                wv_sb = load_weight(wv, DIN, DC, "wv_sb")
                bv_st = const.tile([1, DC], F32)
                nc.sync.dma_start(bv_st[0:1, :], bv[:][None, :])
                bv_bc = const.tile([P, DC], F32)
                nc.gpsimd.partition_broadcast(bv_bc[:], bv_st[0:1, :])
                wo_sb = load_weight(wo, DC, DOUT, "wo_sb")
            for kt in range(4 * st_i, 4 * st_i + 4):
                scores_exp(0, 0, kt)
            if st_i == 3:
                for kt in range(0, 4):
                    scores_exp(0, 1, kt)
        # Phase C: V-proj groups; consume hp0 via AV-T as V chunks land,
        # keep the exp stream fed with hp1 (then hp2) scores.
        for st_i in range(4):
            vproj_chunk(st_i)
            for kt in range(4 * st_i, 4 * st_i + 4):
                av_t(0, 0, kt)
            nxt = [(1, kt) for kt in range(4 * st_i + 4, 4 * st_i + 8)
                   if kt < 16] or [(2, kt) for kt in range(4 * st_i - 12,
                                                           4 * st_i - 8)]
            for hp_n, kt in nxt:
                scores_exp(0, hp_n, kt)
        # Phase D: finish sqt0 (hp1..hp3), qproj(1) as filler.
        qproj_load(1)
        filler = deque()
        for do in range(NDO):
            filler.append((qproj_chain, (1, do)))
        norm_transp(0, 0)
        for kt in range(NKT):
            av_t(0, 1, kt)
            if kt >= 4 and kt < 16:
                scores_exp(0, 2, kt)
            if kt % 4 == 1 and filler:
                f, a = filler.popleft()
                f(*a)
        norm_transp(0, 1)
        for kt in range(NKT):
            av_t(0, 2, kt)
            scores_exp(0, 3, kt)
            if kt % 4 == 1 and filler:
                f, a = filler.popleft()
                f(*a)
        norm_transp(0, 2)
        for kt in range(NKT):
            av_t(0, 3, kt)
            if kt % 4 == 1 and filler:
                f, a = filler.popleft()
                f(*a)
        norm_transp(0, 3)
        while filler:
            f, a = filler.popleft()
            f(*a)

        # Phase E: steady sqt loop; previous sqt's out-proj and next sqt's
        # Q-proj interleaved as PE filler between score/exp pairs.
        for sqt in range(1, NSQT):
            last = sqt == NSQT - 1
            if not last:
                qproj_load(sqt + 1)
            filler = deque()
            if not last:
                for do in range(NDO):
                    filler.append((qproj_chain, (sqt + 1, do)))
            for do in range(DOUT // 512):
                for sc in range(NSQC):
                    filler.append((oproj_chain, (sqt - 1, do, sc)))
            for hp in range(H // 2):
                for kt in range(NKT):
                    scores_exp(sqt, hp, kt)
                    if kt > 0:
                        av_t(sqt, hp, kt - 1)
                    if kt % 4 == 1 and filler:
                        f, a = filler.popleft()
                        f(*a)
                av_t(sqt, hp, NKT - 1)
                # on the last sqt, chase each output transpose with its
                # out-proj chains to shorten the tail
                norm_transp(sqt, hp, oproj_after_qc=(last and hp == 3))
            while filler:
                f, a = filler.popleft()
                f(*a)

    nc.compile()
    return nc


# ---------------------------------------------------------------------------
# Host-side wrapper: shard across 8 NeuronCores, run SPMD, gather.
# Core c handles batch b = c // 2 and head-group g = c % 2 (8 of 16 heads,
# i.e. columns [g*512, (g+1)*512) of Wq/Wk/Wv and rows of Wo).
# ---------------------------------------------------------------------------

import numpy as np
import ml_dtypes

from concourse.bass_utils import run_bass_kernel_spmd

_NC = None
_BF16 = ml_dtypes.bfloat16


def _get_nc():
    global _NC
    if _NC is None:
        _NC = build_mha_core(S=2048, DIN=1024, DC=512, DOUT=1024, H=8,
                             depth=64, num_devices=8)
    return _NC


def _in_maps(q, k, v, Wq, bq, Wk, bk, Wv, bv, Wo, bo):
    f32 = np.float32
    maps = []
    qb = [np.ascontiguousarray(np.asarray(q[b], dtype=f32).astype(_BF16))
          for b in range(4)]
    kb = [np.ascontiguousarray(np.asarray(k[b], dtype=f32).astype(_BF16))
          for b in range(4)]
    vb = [np.ascontiguousarray(np.asarray(v[b], dtype=f32).astype(_BF16))
          for b in range(4)]
    Wq = np.asarray(Wq, dtype=f32)
    Wk = np.asarray(Wk, dtype=f32)
    Wv = np.asarray(Wv, dtype=f32)
    Wo = np.asarray(Wo, dtype=f32)
    for c in range(8):
        b, g = c // 2, c % 2
        sl = slice(g * 512, (g + 1) * 512)
        maps.append({
            "xq": qb[b],
            "xk": kb[b],
            "xv": vb[b],
            "wq": np.ascontiguousarray(Wq[:, sl].astype(_BF16)),
            "wk": np.ascontiguousarray(Wk[:, sl].astype(_BF16)),
            "wv": np.ascontiguousarray(Wv[:, sl].astype(_BF16)),
            "wo": np.ascontiguousarray(Wo[sl, :].astype(_BF16)),
            "bq": np.ascontiguousarray(bq[sl], dtype=f32),
            "bk": np.ascontiguousarray(bk[sl], dtype=f32),
            "bv": np.ascontiguousarray(bv[sl], dtype=f32),
        })
    return maps


def _gather(results, bo):
    out = np.empty((4, 2048, 1024), dtype=np.float32)
    bo32 = np.asarray(bo, dtype=np.float32)
    for b in range(4):
        out[b] = results[2 * b]["out"] + results[2 * b + 1]["out"] + bo32
    return out


def kernel(q, k, v, Wq, bq, Wk, bk, Wv, bv, Wo, bo, _trace=False):
    nc = _get_nc()
    res = run_bass_kernel_spmd(
        nc, _in_maps(q, k, v, Wq, bq, Wk, bk, Wv, bv, Wo, bo),
        core_ids=list(range(8)), trace=_trace)
    out = _gather(res.results, bo)
    if _trace:
        kernel.last_results = res
    return out
